# revision 25
# baseline (speedup 1.0000x reference)
"""Trainium2 Bass kernel for nn_MetricLearningLoss (N=8192, D=128, C=100 classes).

Math: with d2[i,j] = ||x_i - x_j||^2,
  same_sum = sum_{l_i==l_j} d2 = sum_c [ 2*n_c*SS_c - 2*||M_c||^2 ]
  total_sum = sum_{i,j} d2     = 2*N*SS_tot - 2*||M_tot||^2
  loss = -0.5*same_sum/(2*sigma^2) + 0.5*(total_sum - same_sum)/(2*omega^2)
where per class c: n_c = count, M_c = sum of member rows, SS_c = sum of member
squared norms.  (The reference's max(d2, 0) clamp only affects fp32 noise on
the diagonal, ~1e-8 relative.)

Each of the 8 cores reduces its 1024-row shard to a [100, 130] block
[M_c | SS_c | n_c] via one-hot matmuls on the PE, a 52KB AllGather combines the
shards, and every core computes the same final scalar on-device.

Raw Bass (no TileContext): this container's walrus rejects the
EVENT_SEMAPHORE_RANGE_CLEAR raw-ISA op that TileContext's exit always emits,
so synchronization is explicit semaphores below.
"""

from contextlib import ExitStack

import numpy as np

import concourse.bass as bass
import concourse.mybir as mybir
from concourse.bass_utils import run_bass_kernel_spmd

N, D, C = 8192, 128, 100
CORES = 8
ROWS = N // CORES  # 1024 rows per core
KT = ROWS // 128   # 8 k-tiles of 128 rows
SIGMA, OMEGA = 0.2, 1.0
# loss = C_SS*SS_tot + C_MSQ*||M_tot||^2 + C_SAME*same_sum
C_SAME = -(0.5 / (2 * SIGMA**2) + 0.5 / (2 * OMEGA**2))  # -6.5
C_SS = (0.5 / (2 * OMEGA**2)) * 2 * N                    # 4096
C_MSQ = -(0.5 / (2 * OMEGA**2)) * 2                      # -0.5
F32 = mybir.dt.float32
I32 = mybir.dt.int32
FW = D + 2  # 130: [M_c (128) | SS_c | n_c]


def build(debug=False, front_only=False):
    nc = bass.Bass()
    x_in = nc.dram_tensor("x", [ROWS, D], F32, kind="ExternalInput")
    lab_in = nc.dram_tensor("labels", [ROWS], I32, kind="ExternalInput")
    loss_out = nc.dram_tensor("loss", [1], F32, kind="ExternalOutput")
    if debug:
        dbg = {
            "dbg_iota": nc.dram_tensor("dbg_iota", [128, C], F32, kind="ExternalOutput"),
            "dbg_lab": nc.dram_tensor("dbg_lab", [128, KT], F32, kind="ExternalOutput"),
            "dbg_h0": nc.dram_tensor("dbg_h0", [128, C], F32, kind="ExternalOutput"),
            "dbg_aux": nc.dram_tensor("dbg_aux", [128, 2 * KT], F32, kind="ExternalOutput"),
            "dbg_partial": nc.dram_tensor("dbg_partial", [C, FW], F32, kind="ExternalOutput"),
            "dbg_gath": nc.dram_tensor("dbg_gath", [C, CORES * FW], F32, kind="ExternalOutput"),
            "dbg_S": nc.dram_tensor("dbg_S", [C, FW], F32, kind="ExternalOutput"),
            "dbg_t": nc.dram_tensor("dbg_t", [1, FW], F32, kind="ExternalOutput"),
            "dbg_S_raw": nc.dram_tensor("dbg_S_raw", [C, FW], F32, kind="ExternalOutput"),
            "dbg_nss": nc.dram_tensor("dbg_nss", [C, 1], F32, kind="ExternalOutput"),
            "dbg_rq": nc.dram_tensor("dbg_rq", [C, 1], F32, kind="ExternalOutput"),
            "dbg_sub": nc.dram_tensor("dbg_sub", [C, 1], F32, kind="ExternalOutput"),
        }
    cc_in = nc.dram_tensor("cc_in", [C, FW], F32)
    cc_out = nc.dram_tensor("cc_out", [CORES * C, FW], F32, addr_space="Shared")

    add = mybir.AluOpType.add
    mult = mybir.AluOpType.mult
    is_equal = mybir.AluOpType.is_equal
    X = mybir.AxisListType.X

    with ExitStack() as ctx:
        def sb(name, shape, dtype=F32):
            return ctx.enter_context(nc.sbuf_tensor(name, shape, dtype))

        iota_i = sb("iota_i", [128, C], I32)
        iota_f = sb("iota_f", [128, C])
        lab_i = sb("lab_i", [128, KT], I32)
        lab_f = sb("lab_f", [128, KT])
        # tile-major: row t*128+p of the shard at [p, t*D:(t+1)*D]
        x_all = sb("x_all", [128, KT * D])
        aux = sb("aux", [128, 2 * KT])        # per k-tile [sq | 1] column pairs
        sqall = sb("sqall", [128, KT * D])    # x_all squared elementwise
        hts = [sb(f"ht{t}", [128, C]) for t in range(KT)]
        partial = sb("partial", [128, FW])    # this core's [M | SS | n]
        gath = sb("gath", [128, CORES * FW])  # all 8 cores' partials
        S = sb("S", [128, FW])                # summed over cores
        S_copy = sb("S_copy", [128, FW]) if debug else None
        nss = sb("nss", [128, 1])
        tmpm = sb("tmpm", [128, D])
        rq = sb("rq", [128, 1])
        sub = sb("sub", [128, 1])
        t_sb = sb("t_sb", [128, FW])          # [M_tot | SS_tot | same_sum]
        tss = sb("tss", [128, 1])
        tmpt = sb("tmpt", [128, D])
        rqt = sb("rqt", [128, 1])
        part_a = sb("part_a", [128, 1])
        loss_sb = sb("loss_sb", [128, 1])

        px = ctx.enter_context(nc.psum_tensor([128, D], F32))
        pa = ctx.enter_context(nc.psum_tensor([128, 2], F32))

        dsem = ctx.enter_context(nc.semaphore("dsem"))  # misc DMA completions
        xsem_a = ctx.enter_context(nc.semaphore("xsem_a"))  # x tiles 0..3 DMA
        xsem_b = ctx.enter_context(nc.semaphore("xsem_b"))  # x tiles 4..7 DMA
        vsem = ctx.enter_context(nc.semaphore("vsem"))  # DVE progress
        psem = ctx.enter_context(nc.semaphore("psem"))  # PE progress
        asem = ctx.enter_context(nc.semaphore("asem"))  # ACT progress
        csem = ctx.enter_context(nc.semaphore("csem"))  # collective done
        gsem = ctx.enter_context(nc.semaphore("gsem"))  # gpsimd iota done

        block = ctx.enter_context(nc.Block())

        @block.vector
        def _(v):
            # NOTE: same-engine dependent ops need explicit waits — the DVE
            # pipeline is deep and back-to-back instructions do not see each
            # other's writes (sim race detector confirms).
            v.wait_ge(dsem, 16)
            v.tensor_copy(lab_f[:], lab_i[:]).then_inc(vsem, 1)     # 1
            v.wait_ge(gsem, 1)
            v.tensor_copy(iota_f[:], iota_i[:]).then_inc(vsem, 1)   # 2
            v.wait_ge(vsem, 2)                        # RAW iota_f/lab_f
            for t in range(KT):                       # one-hots first: PE can
                v.tensor_scalar(                      # start before x loads
                    hts[t][:], iota_f[:], lab_f[:, t:t + 1], None, is_equal,
                ).then_inc(vsem, 1)                                 # 3+t
            v.memset(aux[:], 1.0).then_inc(vsem, 1)                 # 11
            v.wait_ge(xsem_a, 16)
            v.wait_ge(xsem_b, 16)
            v.tensor_tensor(sqall[:], x_all[:], x_all[:], mult).then_inc(vsem, 1)  # 12
            v.wait_ge(vsem, 12)                       # RAW sqall, WAW aux memset
            v.tensor_reduce(                          # sq cols (even) of aux
                out=aux[:].rearrange("p (t two) -> p t two", two=2)[:, :, 0],
                in_=sqall[:].rearrange("p (t d) -> p t d", d=D),
                axis=X, op=add,
            ).then_inc(vsem, 1)                                     # 13
            if front_only:
                nc._v_sc_done = nc._v_all_done = 13
                return
            v.wait_ge(dsem, 48)
            v.tensor_reduce(
                out=S[0:C, :], in_=gath[0:C, :].rearrange("p (r f) -> p f r", r=CORES),
                axis=X, op=add,
            ).then_inc(vsem, 1)                                     # 14
            vc = 14
            if debug:
                v.wait_ge(vsem, vc)                   # RAW on S
                v.tensor_copy(S_copy[0:C, :], S[0:C, :]).then_inc(vsem, 1)
                vc += 1
            # s_c = 2*(n_c*SS_c - ||M_c||^2) into S[:, D+1]
            v.wait_ge(vsem, 14)                       # RAW on S
            v.tensor_tensor(nss[0:C, :], S[0:C, D + 1:D + 2], S[0:C, D:D + 1],
                            mult).then_inc(vsem, 1)
            v.tensor_tensor(tmpm[0:C, :], S[0:C, 0:D], S[0:C, 0:D],
                            mult).then_inc(vsem, 1)
            vc += 2
            v.wait_ge(vsem, vc)                       # RAW on tmpm
            v.tensor_reduce(out=rq[0:C, :], in_=tmpm[0:C, :], axis=X,
                            op=add).then_inc(vsem, 1)
            vc += 1
            v.wait_ge(vsem, vc)                       # RAW on rq (and nss)
            v.tensor_tensor(sub[0:C, :], nss[0:C, :], rq[0:C, :],
                            mybir.AluOpType.subtract).then_inc(vsem, 1)
            vc += 1
            v.wait_ge(vsem, vc)                       # RAW on sub
            v.tensor_scalar(S[0:C, D + 1:D + 2], sub[0:C, :], 2.0, None,
                            mult).then_inc(vsem, 1)
            vc += 1
            nc._v_sc_done = vc                        # gpsimd colsum waits this
            v.wait_ge(gsem, 2)                        # t_sb colsum done (Pool)
            # loss = C_SS*SS_tot + C_MSQ*||M_tot||^2 + C_SAME*same_sum
            v.tensor_scalar(tss[0:1, :], t_sb[0:1, D:D + 1], float(C_SS), None,
                            mult).then_inc(vsem, 1)
            v.tensor_tensor(tmpt[0:1, :], t_sb[0:1, 0:D], t_sb[0:1, 0:D],
                            mult).then_inc(vsem, 1)
            vc += 2
            v.wait_ge(vsem, vc)                       # RAW on tmpt
            v.tensor_reduce(out=rqt[0:1, :], in_=tmpt[0:1, :], axis=X,
                            op=add).then_inc(vsem, 1)
            vc += 1
            v.wait_ge(vsem, vc)                       # RAW on rqt (and tss)
            v.tensor_scalar(part_a[0:1, :], rqt[0:1, :], float(C_MSQ),
                            tss[0:1, :], mult, add).then_inc(vsem, 1)
            vc += 1
            v.wait_ge(vsem, vc)                       # RAW on part_a
            v.tensor_scalar(
                loss_sb[0:1, :], t_sb[0:1, D + 1:D + 2], float(C_SAME),
                part_a[0:1, :], mult, add,
            ).then_inc(vsem, 1)
            vc += 1
            nc._v_all_done = vc                       # sync loss DMA waits this

        HALF = KT // 2

        @block.sync
        def _(sync):
            # host pre-transposes the shard to tile-major, so this is a
            # contiguous [128, KT] load with lab_i[p, t] = labels[t*128+p]
            sync.dma_start(
                out=lab_i[:], in_=lab_in[:].rearrange("(p t) -> p t", t=KT)
            ).then_inc(dsem, 16)                                    # dsem 16
            sync.dma_start(
                out=x_all[:, 0:HALF * D].rearrange("p (t d) -> p t d", d=D),
                in_=x_in[0:HALF * 128, :].rearrange("(t p) d -> p t d", p=128),
            ).then_inc(xsem_a, 16)
            sync.wait_ge(asem, 2)
            sync.dma_start(out=cc_in[:], in_=partial[0:C, :]).then_inc(dsem, 16)  # 32
            if front_only:
                sync.dma_start(out=loss_out[:], in_=partial[0:1, 0:1]).then_inc(dsem, 16)
                sync.wait_ge(dsem, 48)
                return
            sync.wait_ge(csem, 1)
            sync.dma_start(
                out=gath[0:C, :].rearrange("p (r f) -> p r f", r=CORES),
                in_=cc_out[:].rearrange("(r p) f -> p r f", r=CORES),
            ).then_inc(dsem, 16)                                    # dsem 48
            sync.wait_ge(vsem, nc._v_all_done)
            sync.dma_start(out=loss_out[:], in_=loss_sb[0:1, 0:1]).then_inc(dsem, 16)
            nd = 64
            if debug:
                for name, src in [
                    ("dbg_iota", iota_f[:]), ("dbg_lab", lab_f[:]),
                    ("dbg_h0", hts[0][:]), ("dbg_aux", aux[:]),
                    ("dbg_partial", partial[0:C, :]), ("dbg_gath", gath[0:C, :]),
                    ("dbg_S", S[0:C, :]), ("dbg_t", t_sb[0:1, :]),
                    ("dbg_S_raw", S_copy[0:C, :]), ("dbg_nss", nss[0:C, :]),
                    ("dbg_rq", rq[0:C, :]), ("dbg_sub", sub[0:C, :]),
                ]:
                    sync.dma_start(out=dbg[name][:], in_=src).then_inc(dsem, 16)
                    nd += 16
            sync.wait_ge(dsem, nd)

        @block.gpsimd
        def _(g):
            g.iota(iota_i[:], pattern=[[1, C]], base=0, channel_multiplier=0
                   ).then_inc(gsem, 1)
            if front_only:
                return
            g.wait_ge(dsem, 32)
            g.collective_compute(
                "AllGather", mybir.AluOpType.bypass,
                replica_groups=[list(range(CORES))],
                ins=[cc_in[:]], outs=[cc_out[:]],
            ).then_inc(csem, 1)
            # t_sb[0, :] = sum over classes of S = [M_tot | SS_tot | same_sum]
            g.wait_ge(vsem, nc._v_sc_done)
            g.tensor_reduce(out=t_sb[0:1, :], in_=S[0:C, :],
                            axis=mybir.AxisListType.C, op=add).then_inc(gsem, 1)

        @block.tensor
        def _(te):
            te.wait_ge(xsem_a, 16)
            for t in range(KT):
                if t == KT // 2:
                    te.wait_ge(xsem_b, 16)
                te.wait_ge(vsem, 3 + t)               # ht_t done
                te.matmul(px[0:C, :], lhsT=hts[t][:], rhs=x_all[:, t * D:(t + 1) * D],
                          start=(t == 0), stop=(t == KT - 1)).then_inc(psem, 1)
            te.wait_ge(vsem, 13)                      # aux sq column done
            for t in range(KT):                                     # psem 9..16
                te.matmul(pa[0:C, :], lhsT=hts[t][:], rhs=aux[:, 2 * t:2 * t + 2],
                          start=(t == 0), stop=(t == KT - 1)).then_inc(psem, 1)

        @block.scalar
        def _(sc):
            sc.dma_start(
                out=x_all[:, HALF * D:].rearrange("p (t d) -> p t d", d=D),
                in_=x_in[HALF * 128:, :].rearrange("(t p) d -> p t d", p=128),
            ).then_inc(xsem_b, 16)
            sc.wait_ge(psem, 8)
            sc.copy(partial[0:C, 0:D], px[0:C, :]).then_inc(asem, 1)
            sc.wait_ge(psem, 16)
            sc.copy(partial[0:C, D:D + 2], pa[0:C, :]).then_inc(asem, 1)

    return nc


def make_in_maps(outputs, labels):
    x = np.ascontiguousarray(np.asarray(outputs, dtype=np.float32))
    lab = np.ascontiguousarray(np.asarray(labels).astype(np.int32))
    assert x.shape == (N, D) and lab.shape == (N,)
    in_maps = []
    for m in range(CORES):
        shard = lab[m * ROWS:(m + 1) * ROWS]
        # tile-major so the device label load is contiguous: element p*KT+t
        # holds labels[t*128+p], matching x tile t = shard rows t*128..t*128+127
        lab_tm = np.ascontiguousarray(shard.reshape(KT, 128).T).ravel()
        in_maps.append({"x": x[m * ROWS:(m + 1) * ROWS], "labels": lab_tm})
    return in_maps


def run(outputs, labels, **kwargs):
    nc = build()
    in_maps = make_in_maps(outputs, labels)
    return run_bass_kernel_spmd(nc, in_maps, core_ids=list(range(CORES)), **kwargs)


def kernel(outputs, labels):
    res = run(outputs, labels)
    return np.array(res.results[0]["loss"][0], dtype=np.float32).reshape(())
